# revision 1
# baseline (speedup 1.0000x reference)
"""Trainium2 Bass kernel for DiagonalVectorSpinGlassAttention.

Math (derived analytically from the reference; verified vs jax.jacrev to
rel err 6e-7): with xs = per-head unit-normalized x, for each head h

    q = xs_flat @ Wq_h^T          k = xs_flat @ Wk_h^T      (n, 64)
    P = softmax(q k^T, rows)
    out[:, h*64:(h+1)*64] = (P @ k) @ Wq_hh + (P^T @ q) @ Wk_hh + c0 * xs_h

where Wq_hh / Wk_hh are the (64, 64) diagonal blocks of W_qk that map head-h
input columns, and c0 = 0.5 / v with v = (0.5 + sqrt(1.25)) / 2 (the
discriminant of the reference's quadratic collapses to 0.25 + beta^2 * |x|^2
and |x|^2 == 1 after normalization, making the local term a constant scale).
The mask is all-True in this problem, so it is a no-op.

Sharding: head-parallel over 8 cores, 2 head-slots per core (cores 0-3 get 2
real heads, cores 4-7 get 1 real head + 1 dummy slot).
"""

import numpy as np

import concourse.bass as bass
import concourse.tile as tile
from concourse import mybir
from concourse import bass_utils
from concourse.masks import make_identity

H, D = 12, 64
N = 1024
DIM = H * D  # 768
P = 128
NT = N // P  # 8 token tiles
NC = DIM // P  # 6 contraction tiles
NCORES = 8
SLOTS = 2
C0 = np.float32(0.5 / ((0.5 + np.sqrt(1.25)) / 2.0))  # 0.618034
F32 = mybir.dt.float32

# head assignment: slot 0 = heads 0..7, slot 1 = heads 8..11 on cores 0..3
HEAD_MAP = [[c, c + 8 if c < 4 else -1] for c in range(NCORES)]

_cache = {}


def _ts(i, size):
    return slice(i * size, (i + 1) * size)


def _build_kernel_body(tc):
    import os
    STAGE = int(os.environ.get("K_STAGE", "9"))
    REPS = int(os.environ.get("K_REPS", "1"))
    ATTN = os.environ.get("K_ATTN", "bf16")  # bf16 | f32r | f32
    PROJ = os.environ.get("K_PROJ", "f32r")  # f32r | f32
    BF16 = mybir.dt.bfloat16
    F32R = mybir.dt.float32r
    ADT = {"bf16": BF16, "f32r": F32R, "f32": F32}[ATTN]

    def acast(ap):
        return ap

    def pcast(ap):
        return ap

    nc = tc.nc
    Exp = mybir.ActivationFunctionType.Exp
    mult = mybir.AluOpType.mult
    add = mybir.AluOpType.add

    at_d = nc.dram_tensor("at", (DIM, N), F32, kind="ExternalInput").ap()
    wqk_d = nc.dram_tensor("wqk", (SLOTS, DIM, 128), F32, kind="ExternalInput").ap()
    whh_d = nc.dram_tensor("whh", (SLOTS, 64, 128), F32, kind="ExternalInput").ap()
    ats_d = nc.dram_tensor("ats", (SLOTS, 64, N), F32, kind="ExternalInput").ap()
    c0i_d = nc.dram_tensor("c0i", (64, 64), F32, kind="ExternalInput").ap()
    out_d = nc.dram_tensor("out", (SLOTS, N, 64), F32, kind="ExternalOutput").ap()

    import contextlib

    ctx = contextlib.ExitStack()
    with ctx:
        const = ctx.enter_context(tc.tile_pool(name="const", bufs=1))
        wpool = ctx.enter_context(tc.tile_pool(name="wpool", bufs=2))
        spool = ctx.enter_context(tc.tile_pool(name="spool", bufs=2))
        small = ctx.enter_context(tc.tile_pool(name="small", bufs=3))
        pp_s = ctx.enter_context(tc.tile_pool(name="pp_s", bufs=2, space="PSUM"))
        pp_uw = ctx.enter_context(tc.tile_pool(name="pp_uw", bufs=2, space="PSUM"))
        pp_sm = ctx.enter_context(tc.tile_pool(name="pp_sm", bufs=1, space="PSUM"))

        # constants: 128x128 identity (for PE transpose), c0*I_64
        ident = const.tile([P, P], ADT)
        make_identity(nc, ident[:])
        c0i_sb = const.tile([64, 64], F32)
        nc.sync.dma_start(c0i_sb[:], c0i_d)

        # A^T: (768, 1024) -> 6 tiles of (128, 1024) so projection can start
        # as soon as the first contraction tile lands
        at3 = at_d.rearrange("(c p) m -> p c m", p=P)
        at_tiles = []
        at_mm_tiles = []
        for c in range(NC):
            at_c = const.tile([P, N], F32, tag=f"at{c}")
            nc.sync.dma_start(at_c[:], at3[:, c, :])
            at_tiles.append(at_c)
            if PROJ == "f32r":
                at_r = const.tile([P, N], F32R, tag=f"atr{c}")
                nc.vector.tensor_copy(at_r[:], at_c[:])
                at_mm_tiles.append(at_r)
            else:
                at_mm_tiles.append(at_c)

        for s in [s_ for _ in range(REPS) for s_ in range(SLOTS)]:
            # ---- per-slot weights ----
            wqk_sb = wpool.tile([P, NC, 128], F32, tag="wqk")
            nc.sync.dma_start(wqk_sb[:], wqk_d[s].rearrange("(c p) m -> p c m", p=P))
            if PROJ == "f32r":
                wqk_mm = wpool.tile([P, NC, 128], F32R, tag="wqk_r")
                nc.vector.tensor_copy(wqk_mm[:], wqk_sb[:])
            else:
                wqk_mm = wqk_sb
            whh_sb = wpool.tile([64, 128], F32, tag="whh")
            nc.sync.dma_start(whh_sb[:], whh_d[s])
            if ADT != F32:
                whh_r = wpool.tile([64, 128], ADT, tag="whh_r")
                nc.vector.tensor_copy(whh_r[:], whh_sb[:])
            else:
                whh_r = whh_sb
            atsT_sb = wpool.tile([64, N], F32, tag="ats")
            nc.sync.dma_start(atsT_sb[:], ats_d[s])

            # ---- projection: qkT = [q^T; k^T] (128, 1024) ----
            qkT = spool.tile([P, N], ADT, tag="qkT")
            ps_qk = pp_s.tile([P, N], F32, tag="ps_s")
            for hf in range(2):
                for c in range(NC):
                    nc.tensor.matmul(
                        ps_qk[:, _ts(hf, 512)],
                        lhsT=wqk_mm[:, c, :],
                        rhs=at_mm_tiles[c][:, _ts(hf, 512)],
                        start=(c == 0),
                        stop=(c == NC - 1),
                    )
            nc.vector.tensor_copy(qkT[:], ps_qk[:])
            # swapped copy [k^T; q^T] so both q^T and k^T exist at partitions 0-63
            kqT = spool.tile([P, N], ADT, tag="kqT")
            nc.sync.dma_start(kqT[0:64, :], qkT[64:128, :])
            nc.sync.dma_start(kqT[64:128, :], qkT[0:64, :])

            if STAGE == 1:
                dbg = small.tile([P, 64], F32, tag="out_t")
                nc.vector.tensor_copy(dbg[:], kqT[:, 0:64])
                nc.sync.dma_start(out_d[s, 0:P, :], dbg[:])
                continue

            # ---- token-layout q|k via PE transpose: qk_tok (128, 8, 128) ----
            qk_tok = spool.tile([P, NT, P], ADT, tag="qk_tok")
            for t in range(NT):
                ps_tp = pp_s.tile([P, P], ADT, tag="ps_s")
                nc.tensor.transpose(ps_tp[:], qkT[:, _ts(t, P)], ident[:])
                nc.vector.tensor_copy(qk_tok[:, t, :], ps_tp[:])

            if STAGE == 2:
                dbg = small.tile([P, 64], F32, tag="out_t")
                nc.vector.tensor_copy(dbg[:], qk_tok[:, 0, 0:64])
                nc.sync.dma_start(out_d[s, 0:P, :], dbg[:])
                continue

            # ---- E2 = exp(k q^T) (j on partitions) ----
            e2 = spool.tile([P, NT, N], ADT, tag="e2")
            for t in range(NT):
                ps_s2 = pp_s.tile([P, N], F32, tag="ps_s")
                nc.tensor.matmul(ps_s2[:, 0:512], lhsT=acast(kqT[0:64, _ts(t, P)]),
                                 rhs=acast(qkT[0:64, 0:512]), start=True, stop=True)
                nc.tensor.matmul(ps_s2[:, 512:1024], lhsT=acast(kqT[0:64, _ts(t, P)]),
                                 rhs=acast(qkT[0:64, 512:1024]), start=True, stop=True)
                nc.scalar.activation(e2[:, t, :], ps_s2[:], Exp)

            if STAGE == 3:
                dbg = small.tile([P, 64], F32, tag="out_t")
                nc.vector.tensor_copy(dbg[:], e2[:, 0, 0:64])
                nc.sync.dma_start(out_d[s, 0:P, :], dbg[:])
                continue

            # ---- E1 = exp(q k^T) (i on partitions), rowsum r via accum ----
            e1 = spool.tile([P, NT, N], ADT, tag="e1")
            racc = small.tile([P, NT], F32, tag="racc")
            for t in range(NT):
                ps_s1 = pp_s.tile([P, N], F32, tag="ps_s")
                nc.tensor.matmul(ps_s1[:, 0:512], lhsT=acast(qkT[0:64, _ts(t, P)]),
                                 rhs=acast(kqT[0:64, 0:512]), start=True, stop=True)
                nc.tensor.matmul(ps_s1[:, 512:1024], lhsT=acast(qkT[0:64, _ts(t, P)]),
                                 rhs=acast(kqT[0:64, 512:1024]), start=True, stop=True)
                nc.scalar.activation(e1[:, t, :], ps_s1[:], Exp,
                                     accum_out=racc[:, t : t + 1])

            # recip = 1/r  (token-partition layout (128, 8))
            recip = small.tile([P, NT], F32, tag="recip")
            nc.vector.reciprocal(recip[:], racc[:])

            # q' = q / r (token layout)
            qp = spool.tile([P, NT, 64], ADT, tag="qp")
            for t in range(NT):
                nc.vector.tensor_scalar_mul(qp[:, t, :], qk_tok[:, t, 0:64],
                                            recip[:, t : t + 1])

            if STAGE == 4:
                dbg = small.tile([P, 64], F32, tag="out_t")
                nc.vector.tensor_copy(dbg[:], qp[:, 0, :])
                nc.sync.dma_start(out_d[s, 0:P, :], dbg[:])
                continue

            # ---- u_raw^T = k^T E2 (accumulate over j tiles) -> (64, 1024) ----
            uT = spool.tile([64, N], ADT, tag="uT")
            for hf in range(2):
                ps_u = pp_uw.tile([64, 512], F32, tag="ps_uw")
                for t in range(NT):
                    nc.tensor.matmul(ps_u[:], lhsT=acast(qk_tok[:, t, 64:128]),
                                     rhs=acast(e2[:, t, _ts(hf, 512)]),
                                     start=(t == 0), stop=(t == NT - 1))
                nc.vector.tensor_copy(uT[:, _ts(hf, 512)], ps_u[:])

            # ---- w^T = q'^T E1 (accumulate over i tiles) -> (64, 1024) ----
            wT = spool.tile([64, N], ADT, tag="wT")
            for hf in range(2):
                ps_w = pp_uw.tile([64, 512], F32, tag="ps_uw")
                for t in range(NT):
                    nc.tensor.matmul(ps_w[:], lhsT=acast(qp[:, t, :]),
                                     rhs=acast(e1[:, t, _ts(hf, 512)]),
                                     start=(t == 0), stop=(t == NT - 1))
                nc.vector.tensor_copy(wT[:, _ts(hf, 512)], ps_w[:])

            if STAGE == 5:
                dbg = small.tile([P, 64], F32, tag="out_t")
                nc.vector.scalar_tensor_tensor(dbg[0:64, :], uT[:, 0:64], 1.0,
                                               wT[:, 0:64], mult, add)
                nc.sync.dma_start(out_d[s, 0:64, :], dbg[0:64, :])
                continue

            # ---- final: out_t = (uT_t^T @ Wq_hh) * recip + wT_t^T @ Wk_hh + c0*xs
            for t in range(NT):
                # u-term, unscaled (own PSUM bank)
                ps_fu = pp_sm.tile([P, 64], F32, tag="ps_fu")
                nc.tensor.matmul(ps_fu[:], lhsT=uT[:, _ts(t, P)],
                                 rhs=whh_r[:, 0:64], start=True, stop=True)
                # rest: w-term + c0*xs (own PSUM bank; xs term stays fp32)
                ps_fr = pp_sm.tile([P, 64], F32, tag="ps_fr")
                nc.tensor.matmul(ps_fr[:], lhsT=wT[:, _ts(t, P)],
                                 rhs=whh_r[:, 64:128], start=True, stop=False)
                nc.tensor.matmul(ps_fr[:], lhsT=atsT_sb[:, _ts(t, P)],
                                 rhs=c0i_sb[:], start=False, stop=True)
                out_t = small.tile([P, 64], F32, tag="out_t")
                rest = small.tile([P, 64], F32, tag="rest")
                nc.vector.tensor_copy(rest[:], ps_fr[:])
                nc.vector.tensor_scalar_mul(out_t[:], ps_fu[:],
                                            recip[:, t : t + 1])
                nc.vector.tensor_add(out_t[:], out_t[:], rest[:])
                nc.sync.dma_start(out_d[s, _ts(t, P), :], out_t[:])


def _split_multi_waits(nc, limit=1):
    """The walrus build in this container encodes at most one sync-wait per
    instruction. Move extra waits onto NoOp carrier instructions inserted
    just before the offending instruction on the same engine (semantically
    identical: the engine blocks at the same program point)."""
    n_nop = 0
    for fn in nc.m.functions:
        for blk in fn.blocks:
            il = blk.instructions
            idx = 0
            while idx < len(il):
                inst = il[idx]
                si = inst.sync_info
                if si is not None and len(si.on_wait) > limit:
                    waits = list(si.on_wait)
                    extra, keep = waits[:-limit], waits[-limit:]
                    inst.sync_info = mybir.SyncInfo(
                        on_wait=keep, on_update=list(si.on_update)
                    )
                    for w in extra:
                        nop = mybir.InstNoOp(name=f"waitnop-{n_nop}", ins=[],
                                             outs=[])
                        n_nop += 1
                        nop.engine = inst.engine
                        nop.sync_info = mybir.SyncInfo(on_wait=[w], on_update=[])
                        il.insert(idx, nop)
                        idx += 1
                idx += 1
    return n_nop


def _get_nc(split_waits=True):
    key = ("nc", split_waits)
    if key not in _cache:
        nc = bass.Bass("TRN2", debug=False, target_bir_lowering=False,
                       num_devices=NCORES)
        with tile.TileContext(nc) as tc:
            _build_kernel_body(tc)
        if split_waits:
            _split_multi_waits(nc)
        _cache[key] = nc
    return _cache[key]


def _prep_inputs(x, W_qk):
    x = np.asarray(x, dtype=np.float32)
    W = np.asarray(W_qk, dtype=np.float32)
    n = x.shape[0]
    xh = x.reshape(n, H, D)
    nrm = np.sqrt(np.sum(xh * xh, axis=-1, keepdims=True, dtype=np.float32))
    xh = (xh / nrm).astype(np.float32)
    A = np.ascontiguousarray(xh.reshape(n, DIM))
    AT = np.ascontiguousarray(A.T)  # (768, 1024)

    c0i = (C0 * np.eye(64, dtype=np.float32)).astype(np.float32)

    in_maps = []
    for c in range(NCORES):
        wqk = np.zeros((SLOTS, DIM, 128), dtype=np.float32)
        whh = np.zeros((SLOTS, 64, 128), dtype=np.float32)
        ats = np.zeros((SLOTS, 64, N), dtype=np.float32)
        for s in range(SLOTS):
            h = HEAD_MAP[c][s]
            if h < 0:
                h = 0  # dummy slot computes head 0; output ignored
            Wq_h = W[h * D : (h + 1) * D, :]          # (64, 768)
            Wk_h = W[DIM + h * D : DIM + (h + 1) * D, :]
            wqk[s, :, 0:64] = Wq_h.T
            wqk[s, :, 64:128] = Wk_h.T
            whh[s, :, 0:64] = Wq_h[:, h * D : (h + 1) * D]
            whh[s, :, 64:128] = Wk_h[:, h * D : (h + 1) * D]
            ats[s] = AT[h * D : (h + 1) * D, :]
        in_maps.append({
            "at": AT,
            "wqk": np.ascontiguousarray(wqk),
            "whh": np.ascontiguousarray(whh),
            "ats": np.ascontiguousarray(ats),
            "c0i": c0i,
        })
    return in_maps


def kernel(x, mask, W_qk, trace=False):
    nc = _get_nc()
    in_maps = _prep_inputs(x, W_qk)
    res = bass_utils.run_bass_kernel_spmd(
        nc, in_maps, core_ids=list(range(NCORES)), trace=trace
    )
    _cache["last_results"] = res

    out = np.empty((N, DIM), dtype=np.float32)
    for c in range(NCORES):
        for s in range(SLOTS):
            h = HEAD_MAP[c][s]
            if h >= 0:
                out[:, h * D : (h + 1) * D] = res.results[c]["out"][s]
    return out



# revision 7
# speedup vs baseline: 1.2115x; 1.2115x over previous
"""Trainium2 Bass kernel for DiagonalVectorSpinGlassAttention.

Math (derived analytically from the reference; verified vs jax.jacrev): with
xs = per-head unit-normalized x, for each head h

    q = xs_flat @ Wq_h^T          k = xs_flat @ Wk_h^T      (n, 64)
    P = softmax(q k^T, rows)
    out[:, h*64:(h+1)*64] = (P @ k) @ Wq_hh + (P^T @ q) @ Wk_hh + c0 * xs_h

where Wq_hh / Wk_hh are the (64, 64) diagonal blocks of W_qk that map head-h
input columns, and c0 = 0.5 / v with v = (0.5 + sqrt(1.25)) / 2 (the
discriminant of the reference's quadratic collapses to 0.25 + beta^2 * |x|^2
and |x|^2 == 1 after normalization, making the local term a constant scale).
The mask is all-True in this problem, so it is a no-op.

Sharding: head-parallel over 8 cores, 2 head-slots per core (cores 0-3 get 2
real heads, cores 4-7 get 1 real head + 1 dummy slot).

v2 performance restructure vs the original baseline:
  - inputs are float32r (same bits as f32) so projection matmuls run in
    1-cycle/col replay mode with no SBUF cast pass at all
  - input DMA split across both HWDGE queues (sync + scalar) and ordered so
    the first projection matmul starts as early as possible
  - whh shipped pre-duplicated to 128 partitions and ats pre-scaled by c0 in
    bf16, so the final-phase matmuls are all 1-cycle bf16
  - single k-half swap DMA instead of two, single output DMA per slot
  - E2 phase emission interleaved with the uT/wT accumulation chains so the
    PE stays busy while the scalar engine runs the exp activations
"""

import numpy as np

import concourse.bass as bass
import concourse.tile as tile
from concourse import mybir
from concourse import bass_utils
from concourse.masks import make_identity

H, D = 12, 64
N = 1024
DIM = H * D  # 768
P = 128
NT = N // P  # 8 token tiles
NC = DIM // P  # 6 contraction tiles
NCORES = 8
SLOTS = 2
C0 = np.float32(0.5 / ((0.5 + np.sqrt(1.25)) / 2.0))  # 0.618034
F32 = mybir.dt.float32
F32R = mybir.dt.float32r
BF16 = mybir.dt.bfloat16

# head assignment: slot 0 = heads 0..7, slot 1 = heads 8..11 on cores 0..3
HEAD_MAP = [[c, c + 8 if c < 4 else -1] for c in range(NCORES)]

_cache = {}


def _ts(i, size):
    return slice(i * size, (i + 1) * size)


def _build_kernel_body(tc):
    nc = tc.nc
    Exp = mybir.ActivationFunctionType.Exp
    mult = mybir.AluOpType.mult
    add = mybir.AluOpType.add

    at_d = nc.dram_tensor("at", (DIM, N), F32R, kind="ExternalInput").ap()
    wqk_d = nc.dram_tensor("wqk", (SLOTS, DIM, 128), F32R, kind="ExternalInput").ap()
    whh_d = nc.dram_tensor("whh", (SLOTS, P, 128), BF16, kind="ExternalInput").ap()
    ats_d = nc.dram_tensor("ats", (SLOTS, 64, N), BF16, kind="ExternalInput").ap()
    out_d = nc.dram_tensor("out", (SLOTS, N, 64), F32, kind="ExternalOutput").ap()

    import contextlib

    ctx = contextlib.ExitStack()
    with ctx:
        const = ctx.enter_context(tc.tile_pool(name="const", bufs=1))
        wpool = ctx.enter_context(tc.tile_pool(name="wpool", bufs=2))
        spool = ctx.enter_context(tc.tile_pool(name="spool", bufs=2))
        small = ctx.enter_context(tc.tile_pool(name="small", bufs=3))
        pp_e = ctx.enter_context(tc.tile_pool(name="pp_e", bufs=2, space="PSUM"))
        pp_uw = ctx.enter_context(tc.tile_pool(name="pp_uw", bufs=1, space="PSUM"))

        # ---- global inputs ----
        # A^T tiles on the sync queue (projection rhs; also sliced per-head
        # would be core-dependent, so ats comes as its own small input)
        at3 = at_d.rearrange("(c p) m -> p c m", p=P)
        at_tiles = []
        for c in range(NC):
            at_c = const.tile([P, N], F32R, tag=f"at{c}")
            nc.sync.dma_start(at_c[:], at3[:, c, :])
            at_tiles.append(at_c)

        # per-slot weights on the scalar queue (parallel with the at tiles)
        wqk_sb, whh_sb, ats_sb = [], [], []
        for s in range(SLOTS):
            wqk_s = wpool.tile([P, NC, 128], F32R, tag=f"wqk{s}")
            nc.scalar.dma_start(wqk_s[:], wqk_d[s].rearrange("(c p) m -> p c m", p=P))
            whh_s = wpool.tile([P, 128], BF16, tag=f"whh{s}")
            nc.scalar.dma_start(whh_s[:], whh_d[s])
            ats_s = wpool.tile([64, N], BF16, tag=f"ats{s}")
            nc.scalar.dma_start(ats_s[:], ats_d[s])
            wqk_sb.append(wqk_s)
            whh_sb.append(whh_s)
            ats_sb.append(ats_s)

        # 128x128 bf16 identity (PE transposes; [0:64,0:64] = I_64 for ats)
        ident = const.tile([P, P], BF16)
        make_identity(nc, ident[:])

        for s in range(SLOTS):
            wqk_mm = wqk_sb[s]
            whh_r = whh_sb[s]
            atsT_sb = ats_sb[s]

            # ---- projection: qkT = [q^T; k^T] (128, 1024), f32r 1cyc ----
            ps_qk = pp_e.tile([P, N], F32, tag="pe")
            for c in range(NC):
                for hf in range(2):
                    nc.tensor.matmul(
                        ps_qk[:, _ts(hf, 512)],
                        lhsT=wqk_mm[:, c, :],
                        rhs=at_tiles[c][:, _ts(hf, 512)],
                        start=(c == 0),
                        stop=(c == NC - 1),
                    )
            qkT = spool.tile([P, N], BF16, tag="qkT")
            nc.vector.tensor_copy(qkT[:], ps_qk[:])
            # k rows also needed at partitions 0-63: one swap DMA
            k_sb = spool.tile([64, N], BF16, tag="k_sb")
            nc.scalar.dma_start(k_sb[:], qkT[64:128, :])

            # ---- token-layout q|k via PE transpose: qk_tok (128, 8, 128) ----
            qk_tok = spool.tile([P, NT, P], BF16, tag="qk_tok")
            for t in range(NT):
                ps_tp = pp_e.tile([P, P], BF16, tag="pe")
                nc.tensor.transpose(ps_tp[:], qkT[:, _ts(t, P)], ident[:])
                nc.vector.tensor_copy(qk_tok[:, t, :], ps_tp[:])

            # ---- E1 = exp(q k^T) (i on partitions), rowsum r via accum ----
            e1 = spool.tile([P, NT, N], BF16, tag="e1")
            racc = small.tile([P, NT], F32, tag="racc")
            for t in range(NT):
                ps_s1 = pp_e.tile([P, N], F32, tag="pe")
                nc.tensor.matmul(ps_s1[:, 0:512], lhsT=qkT[0:64, _ts(t, P)],
                                 rhs=k_sb[:, 0:512], start=True, stop=True)
                nc.tensor.matmul(ps_s1[:, 512:1024], lhsT=qkT[0:64, _ts(t, P)],
                                 rhs=k_sb[:, 512:1024], start=True, stop=True)
                nc.scalar.activation(e1[:, t, :], ps_s1[:], Exp,
                                     accum_out=racc[:, t : t + 1])

            # recip = 1/r  (token-partition layout (128, 8))
            recip = small.tile([P, NT], F32, tag="recip")
            nc.vector.reciprocal(recip[:], racc[:])

            # q' = q / r (token layout)
            qp = spool.tile([P, NT, 64], BF16, tag="qp")
            for t in range(NT):
                nc.vector.tensor_scalar_mul(qp[:, t, :], qk_tok[:, t, 0:64],
                                            recip[:, t : t + 1])

            # ---- E2 = exp(k q^T) interleaved with the uT and wT chains ----
            # uT = k^T E2 accumulated over j tiles; wT = q'^T E1 over i tiles
            e2 = spool.tile([P, NT, N], BF16, tag="e2")
            ps_u = [pp_uw.tile([64, 512], F32, tag=f"u{hf}", name=f"ps_u{hf}")
                    for hf in range(2)]
            ps_w = [pp_uw.tile([64, 512], F32, tag=f"w{hf}", name=f"ps_w{hf}")
                    for hf in range(2)]
            for t in range(NT):
                ps_s2 = pp_e.tile([P, N], F32, tag="pe")
                nc.tensor.matmul(ps_s2[:, 0:512], lhsT=k_sb[:, _ts(t, P)],
                                 rhs=qkT[0:64, 0:512], start=True, stop=True)
                nc.tensor.matmul(ps_s2[:, 512:1024], lhsT=k_sb[:, _ts(t, P)],
                                 rhs=qkT[0:64, 512:1024], start=True, stop=True)
                nc.scalar.activation(e2[:, t, :], ps_s2[:], Exp)
                for hf in range(2):
                    nc.tensor.matmul(ps_u[hf][:], lhsT=qk_tok[:, t, 64:128],
                                     rhs=e2[:, t, _ts(hf, 512)],
                                     start=(t == 0), stop=(t == NT - 1))
                    nc.tensor.matmul(ps_w[hf][:], lhsT=qp[:, t, :],
                                     rhs=e1[:, t, _ts(hf, 512)],
                                     start=(t == 0), stop=(t == NT - 1))
            uT = spool.tile([64, N], BF16, tag="uT")
            wT = spool.tile([64, N], BF16, tag="wT")
            for hf in range(2):
                nc.vector.tensor_copy(uT[:, _ts(hf, 512)], ps_u[hf][:])
                nc.vector.tensor_copy(wT[:, _ts(hf, 512)], ps_w[hf][:])

            # ---- final: out_t = (uT_t^T @ Wq_hh) * recip + wT_t^T @ Wk_hh
            #                      + (c0*xs)_t  (ats pre-scaled, I_64 rhs)
            out_sb = spool.tile([P, NT, 64], F32, tag="out_sb")
            for t in range(NT):
                ps_fu = pp_uw.tile([P, 64], F32, tag="u0")
                nc.tensor.matmul(ps_fu[:], lhsT=uT[:, _ts(t, P)],
                                 rhs=whh_r[0:64, 0:64], start=True, stop=True)
                ps_fr = pp_uw.tile([P, 64], F32, tag="w0")
                nc.tensor.matmul(ps_fr[:], lhsT=wT[:, _ts(t, P)],
                                 rhs=whh_r[0:64, 64:128], start=True, stop=False)
                nc.tensor.matmul(ps_fr[:], lhsT=atsT_sb[:, _ts(t, P)],
                                 rhs=ident[0:64, 0:64], start=False, stop=True)
                nc.vector.tensor_scalar_mul(out_sb[:, t, :], ps_fu[:],
                                            recip[:, t : t + 1])
                nc.vector.tensor_add(out_sb[:, t, :], out_sb[:, t, :], ps_fr[:])
            nc.sync.dma_start(
                out_d[s].rearrange("(t p) f -> p t f", p=P), out_sb[:]
            )


def _split_multi_waits(nc, limit=1):
    """The walrus build in this container encodes at most one sync-wait per
    instruction. Move extra waits onto NoOp carrier instructions inserted
    just before the offending instruction on the same engine (semantically
    identical: the engine blocks at the same program point)."""
    n_nop = 0
    for fn in nc.m.functions:
        for blk in fn.blocks:
            il = blk.instructions
            idx = 0
            while idx < len(il):
                inst = il[idx]
                si = inst.sync_info
                if si is not None and len(si.on_wait) > limit:
                    waits = list(si.on_wait)
                    extra, keep = waits[:-limit], waits[-limit:]
                    inst.sync_info = mybir.SyncInfo(
                        on_wait=keep, on_update=list(si.on_update)
                    )
                    for w in extra:
                        nop = mybir.InstNoOp(name=f"waitnop-{n_nop}", ins=[],
                                             outs=[])
                        n_nop += 1
                        nop.engine = inst.engine
                        nop.sync_info = mybir.SyncInfo(on_wait=[w], on_update=[])
                        il.insert(idx, nop)
                        idx += 1
                idx += 1
    return n_nop


def _get_nc(split_waits=True):
    key = ("nc", split_waits)
    if key not in _cache:
        nc = bass.Bass("TRN2", debug=False, target_bir_lowering=False,
                       num_devices=NCORES)
        with tile.TileContext(nc) as tc:
            _build_kernel_body(tc)
        if split_waits:
            _split_multi_waits(nc)
        _cache[key] = nc
    return _cache[key]


def _prep_inputs(x, W_qk):
    import ml_dtypes

    bf16 = ml_dtypes.bfloat16
    x = np.asarray(x, dtype=np.float32)
    W = np.asarray(W_qk, dtype=np.float32)
    n = x.shape[0]
    xh = x.reshape(n, H, D)
    nrm = np.sqrt(np.sum(xh * xh, axis=-1, keepdims=True, dtype=np.float32))
    xh = (xh / nrm).astype(np.float32)
    A = np.ascontiguousarray(xh.reshape(n, DIM))
    AT = np.ascontiguousarray(A.T)  # (768, 1024)

    in_maps = []
    for c in range(NCORES):
        wqk = np.zeros((SLOTS, DIM, 128), dtype=np.float32)
        whh = np.zeros((SLOTS, P, 128), dtype=np.float32)
        ats = np.zeros((SLOTS, 64, N), dtype=np.float32)
        for s in range(SLOTS):
            h = HEAD_MAP[c][s]
            if h < 0:
                h = 0  # dummy slot computes head 0; output ignored
            Wq_h = W[h * D : (h + 1) * D, :]          # (64, 768)
            Wk_h = W[DIM + h * D : DIM + (h + 1) * D, :]
            wqk[s, :, 0:64] = Wq_h.T
            wqk[s, :, 64:128] = Wk_h.T
            # duplicated to both partition halves so lhsT slices based at
            # partition 64 can use a same-base rhs
            whh[s, 0:64, 0:64] = Wq_h[:, h * D : (h + 1) * D]
            whh[s, 0:64, 64:128] = Wk_h[:, h * D : (h + 1) * D]
            whh[s, 64:128, :] = whh[s, 0:64, :]
            ats[s] = C0 * AT[h * D : (h + 1) * D, :]
        in_maps.append({
            "at": AT,
            "wqk": np.ascontiguousarray(wqk),
            "whh": np.ascontiguousarray(whh.astype(bf16)),
            "ats": np.ascontiguousarray(ats.astype(bf16)),
        })
    return in_maps


def kernel(x, mask, W_qk, trace=False):
    nc = _get_nc()
    in_maps = _prep_inputs(x, W_qk)
    res = bass_utils.run_bass_kernel_spmd(
        nc, in_maps, core_ids=list(range(NCORES)), trace=trace
    )
    _cache["last_results"] = res

    out = np.empty((N, DIM), dtype=np.float32)
    for c in range(NCORES):
        for s in range(SLOTS):
            h = HEAD_MAP[c][s]
            if h >= 0:
                out[:, h * D : (h + 1) * D] = res.results[c]["out"][s]
    return out
